# revision 7
# baseline (speedup 1.0000x reference)
"""Trainium2 Bass kernel: multi-head attention (dense transformer block).

Computation (per batch b):
    Q = x @ Wq + bq ; K = x @ Wk + bk ; V = x @ Wv + bv        (per head)
    P = exp((Q @ K^T) / sqrt(Dh))                   (no max-subtraction needed:
                                                     scores are O(1) by construction)
    out = sum_h (P @ V / rowsum(P)) @ Wd[h] + bd

Sharding (data + tensor parallel): 8 cores; core c handles batch b = c // 4
and the 4 heads starting at 4*(c % 4). Each core computes a partial output
(bf16, in a [128, lt, m] partition-major DRAM layout for wide DMA lines);
the host unshuffles, sums the 4 partials per batch and adds bd.

Per-core dataflow: heads in 2 pairs stacked on partitions. Stream of 8
"positions" = (pair, sub) x 16 beats; each beat computes one l'-tile of
scores for both heads (PSUM, f32), then exp -> bf16 P-tile (ScalarE
mostly; a few beats per position go to DVE via a Schraudolph int16-bitcast
exp to keep DVE/ACT balanced where PE is light). Attend chains trail their
own sub's beats by ~2 (segments interleaved between beats); the softmax
denominator comes from ones-columns in the V tiles. Normalisation reads
the attend PSUM directly (reciprocal + multiply on DVE). Out-projection is
emitted per l-tile as soon as both pairs' norms for that sub are done, and
the output streams to DRAM in 2-l-tile batches round-robined over the
sync/scalar/gpsimd DMA queues so almost nothing is exposed at the tail.
"""

import os
import sys
from contextlib import ExitStack

import ml_dtypes
import numpy as np

for _p in ("/opt/trn_rl_repo", "/root/.axon_site/_ro/trn_rl_repo"):
    if os.path.isdir(_p) and _p not in sys.path:
        sys.path.append(_p)

import concourse.bass as bass
import concourse.tile as tile
from concourse import bacc, mybir
from concourse.bass import ds
from concourse.bass_utils import run_bass_kernel_spmd
from concourse.masks import make_identity

F32 = mybir.dt.float32
BF16 = mybir.dt.bfloat16
I16 = mybir.dt.int16
BF16_NP = ml_dtypes.bfloat16

# Problem sizes (hardcoded per contract).
DMODEL, HEADS, DHEAD = 1024, 16, 64
B, L = 2, 2048
NCORES = 8
H_PER_CORE = B * HEADS // NCORES          # 4 heads per core
NPAIR = H_PER_CORE // 2                   # head pairs per core
P = 128                                   # partitions
KT = DMODEL // P                          # 8 k-tiles over dmodel
NLT = L // P                              # 16 l-tiles
LCH = 512                                 # sub-chunk width (one psum bank pair)
NSUB = L // LCH                           # 4 sub-chunks
MCH = 512                                 # m-chunk for out-proj
NMC = DMODEL // MCH
PT_BUFS = 40                              # P^T tiles in flight
WCH = 3 * KT * P                          # per-pair packed qkv weight columns
SCALE = 1.0 / np.sqrt(DHEAD)

# Schraudolph exp via int16 bitcast to bf16: i16 = round(x*SCH_A + SCH_B),
# bits(i16) ~ bf16(exp(x*SCALE)).  C=5.6 centers the relative error (~±3%).
SCH_A = (2.0 ** 7 / np.log(2.0)) * SCALE
SCH_B = 127.0 * 2 ** 7 - 5.6

# Beats whose exp runs on DVE instead of ScalarE, per stream position.
DVE_BEATS = {
    (1, 0): (6, 10, 14),
}

EXPF = mybir.ActivationFunctionType.Exp
QUAD_SCORES = True


def build_nc():
    """Build the SPMD Bass program for one core."""
    nc = bacc.Bacc("TRN2", target_bir_lowering=False, debug=False,
                   num_devices=NCORES)

    # Pre-packed per-core inputs (see shard_inputs):
    #   x:    x^T bf16 [dmodel, L] packed [k, sub, kt, l']
    #   wqkv: bf16 [128, pp(2) * i(3: K,Q,V) * kt(8) * 128]
    #   wd2:  bf16 [128, pp(2) * dmodel]
    #   biasp: f32 [128, i(3: K,Q,V) * pp(2)]
    x_d = nc.dram_tensor("x", [P, NSUB * KT * LCH], BF16, kind="ExternalInput").ap()
    wqkv_d = nc.dram_tensor("wqkv", [P, NPAIR * WCH], BF16, kind="ExternalInput").ap()
    wd2_d = nc.dram_tensor("wd2", [P, NPAIR * DMODEL], BF16, kind="ExternalInput").ap()
    biasp_d = nc.dram_tensor("biasp", [P, 3 * NPAIR], F32, kind="ExternalInput").ap()
    # y partition-major: y2[p, lt*DMODEL + m] = y[lt*128 + p, m]
    y_d = nc.dram_tensor("y", [P, NLT * DMODEL], BF16, kind="ExternalOutput").ap()

    with ExitStack() as ctx:
        tc = ctx.enter_context(tile.TileContext(nc))
        _body(nc, tc, ctx, x_d, wqkv_d, wd2_d, biasp_d, y_d)
    nc.compile()
    return nc


def _body(nc, tc, ctx, x_d, wqkv_d, wd2_d, biasp_d, y_d):
    const = ctx.enter_context(tc.tile_pool(name="const", bufs=1))
    sb = ctx.enter_context(tc.tile_pool(name="sb", bufs=1))
    psum = ctx.enter_context(tc.tile_pool(name="psum", bufs=1, space="PSUM"))

    # ACT exp-table warmup off the critical path
    warm = const.tile([P, 1], BF16)
    zz = const.tile([P, 1], F32)
    nc.vector.memset(zz, 0.0)
    nc.scalar.activation(warm, zz, func=EXPF)

    # PE identity (for V transposes) + HAM warm-up matmuls on junk data.
    ident = const.tile([P, P], BF16)
    make_identity(nc, ident)
    for _ in range(6):
        jp = psum.tile([P, 2, P], F32, tag="qkvp", bufs=2, name="jp")
        nc.tensor.matmul(jp[:, 0], lhsT=ident, rhs=ident, start=True, stop=True)
        nc.tensor.matmul(jp[:, 1], lhsT=ident, rhs=ident, start=True, stop=True)

    # ---- input DMAs: weights on the scalar HWDGE queue, x on sync ----
    w_sb = const.tile([P, NPAIR, 3, KT, P], BF16)    # [k, pp, KQV, kt, n]
    wd_sb = const.tile([P, NPAIR, DMODEL], BF16)
    wf = w_sb.rearrange("k pp i kt n -> k (pp i kt n)")
    xt = sb.tile([P, NSUB, KT, LCH], BF16)
    xtf = xt.rearrange("k c kt l -> k (c kt l)")
    CB = KT * LCH
    nc.scalar.dma_start(wf[:, 0:2 * KT * P], wqkv_d[:, 0:2 * KT * P])   # K0+Q0
    nc.sync.dma_start(xtf[:, ds(0, CB // 2)], x_d[:, ds(0, CB // 2)])
    nc.sync.dma_start(xtf[:, ds(CB // 2, CB // 2)], x_d[:, ds(CB // 2, CB // 2)])
    nc.scalar.dma_start(wf[:, ds(2 * KT * P, KT * P)],
                        wqkv_d[:, ds(2 * KT * P, KT * P)])              # V0
    nc.sync.dma_start(xtf[:, ds(CB, CB)], x_d[:, ds(CB, CB)])
    nc.scalar.dma_start(wf[:, ds(WCH, WCH)], wqkv_d[:, ds(WCH, WCH)])   # pair 1
    nc.sync.dma_start(xtf[:, ds(2 * CB, CB)], x_d[:, ds(2 * CB, CB)])
    nc.scalar.dma_start(wd_sb.rearrange("k pp m -> k (pp m)"), wd2_d)
    nc.sync.dma_start(xtf[:, ds(3 * CB, CB)], x_d[:, ds(3 * CB, CB)])

    bias_sb = const.tile([P, 3, NPAIR], F32)
    nc.gpsimd.dma_start(bias_sb.rearrange("k i p -> k (i p)"), biasp_d)

    o_norm = sb.tile([P, NPAIR, L], BF16)

    # per-pair state
    kT_sb = [None] * NPAIR
    qT = [None] * NPAIR
    vT = [None] * NPAIR
    vt = [None] * NPAIR
    pt_tiles = [[[None] * NLT for _ in range(NSUB)] for _ in range(NPAIR)]

    def qkv_chunk(dst, p, i, lc):
        """dst[:, lc*LCH:+LCH] = (W_i^T x^T + b_i) in bf16. i: 0=K, 1=Q, 2=V."""
        ps = psum.tile([P, LCH], F32, tag="qkvp", bufs=2, name="qkvps")
        for kt in range(KT):
            nc.tensor.matmul(
                ps, lhsT=w_sb[:, p, i, kt],
                rhs=xt[:, lc, kt, :],
                start=(kt == 0), stop=(kt == KT - 1))
        nc.vector.tensor_scalar_add(
            dst[:, ds(lc * LCH, LCH)], ps, bias_sb[:, i, p:p + 1])

    def emit_vtr(p, lts):
        """PE-transpose V^T l-tiles into vt[:, lt] = [1*64|V_h0|1*64|V_h1]."""
        for lt in lts:
            tp = psum.tile([P, P], BF16, tag="qkvp", bufs=2, name="tp")
            nc.tensor.transpose(tp, vT[p][:, ds(lt * P, P)], ident)
            out3 = vt[p][:, lt].rearrange("p (i n) -> p i n", n=P)[:, :, DHEAD:P]
            nc.vector.tensor_copy(
                out3, tp.rearrange("p (i n) -> p i n", n=DHEAD))

    def emit_scores(p, s, lt):
        sp = psum.tile([P, 2, LCH], F32, tag="sctr", bufs=2, name="sp")
        if QUAD_SCORES:
            for h in range(2):
                for cq in range(2):
                    nc.tensor.matmul(
                        sp[ds(64 * cq, 64), h, :],
                        lhsT=kT_sb[p][ds(64 * h, 64), ds(lt * P + 64 * cq, 64)],
                        rhs=qT[p][ds(64 * h, 64), ds(s * LCH, LCH)],
                        start=True, stop=True,
                        tile_position=(64 * h, 64 * cq))
        else:
            for h in range(2):
                nc.tensor.matmul(
                    sp[:, h],
                    lhsT=kT_sb[p][ds(64 * h, 64), ds(lt * P, P)],
                    rhs=qT[p][ds(64 * h, 64), ds(s * LCH, LCH)],
                    start=True, stop=True)
        return sp

    def emit_exp(p, s, lt, sp, dve):
        pt = sb.tile([P, 2, LCH], BF16, tag="pt", bufs=PT_BUFS, name="pt")
        src = sp.rearrange("p a b -> p (a b)")
        dst = pt.rearrange("p a b -> p (a b)")
        if dve:
            nc.vector.tensor_scalar(
                dst.bitcast(I16), src, float(SCH_A), float(SCH_B),
                op0=mybir.AluOpType.mult, op1=mybir.AluOpType.add)
        else:
            nc.scalar.activation(dst, src, func=EXPF, scale=float(SCALE))
        pt_tiles[p][s][lt] = pt

    # ---- attend chain bookkeeping ----
    chains = {}

    def att_open(p, s, h):
        chains[(p, s, h)] = {
            "op": psum.tile([P, LCH], F32, tag="op", bufs=2, name="op"),
            "lt": 0}

    def att_seg(p, s, h, upto):
        c = chains[(p, s, h)]
        for lt in range(c["lt"], upto):
            nc.tensor.matmul(
                c["op"], lhsT=vt[p][:, lt, ds(P * h, P)],
                rhs=pt_tiles[p][s][lt][:, h, :],
                start=(lt == 0), stop=(lt == NLT - 1),
                skip_group_check=True)
        c["lt"] = upto

    def att_close(p, s, h):
        att_seg(p, s, h, NLT)
        op = chains.pop((p, s, h))["op"]
        rs = sb.tile([DHEAD, LCH], F32, tag="rs", bufs=2)
        nc.vector.reciprocal_approx_fast(rs, op[0:DHEAD, :])
        nc.vector.tensor_mul(
            o_norm[ds(64 * h, 64), p, ds(s * LCH, LCH)], op[DHEAD:P, :], rs)

    # ---- out-projection + output DMA ----
    ysb_tiles = {}

    def emit_outproj_lt(lt, drain="vector"):
        pr = lt // 2
        if pr not in ysb_tiles:
            ysb_tiles[pr] = sb.tile([P, 2, DMODEL], BF16, tag="ysb", bufs=3,
                                    name="ysb")
        yb = ysb_tiles[pr]
        yps = [psum.tile([P, MCH], F32, tag="qkvp", bufs=2, name="yp")
               for _ in range(NMC)]
        for p in range(NPAIR):
            for mc in range(NMC):
                nc.tensor.matmul(
                    yps[mc], lhsT=o_norm[:, p, ds(lt * P, P)],
                    rhs=wd_sb[:, p, ds(mc * MCH, MCH)],
                    start=(p == 0), stop=(p == NPAIR - 1),
                    skip_group_check=True)
        for mc in range(NMC):
            dstc = yb[:, lt % 2, ds(mc * MCH, MCH)]
            if drain == "scalar":
                nc.scalar.copy(dstc, yps[mc])
            else:
                nc.vector.tensor_copy(dstc, yps[mc])

    def emit_ydma(pr, queue):
        yb = ysb_tiles.pop(pr)
        dst = y_d[:, ds(pr * 2 * DMODEL, 2 * DMODEL)]
        eng = {"sync": nc.sync, "scalar": nc.scalar, "gpsimd": nc.gpsimd}[queue]
        eng.dma_start(dst, yb.rearrange("p a b -> p (a b)"))

    def new_pair(p):
        vT[p] = sb.tile([P, L], BF16, tag="vT", bufs=1, name="vT")
        vt[p] = sb.tile([P, NLT, 2 * P], BF16, tag="vt", bufs=NPAIR, name="vt")
        nc.vector.memset(vt[p][:, :, 0:DHEAD], 1.0)
        nc.vector.memset(vt[p][:, :, P:P + DHEAD], 1.0)

    def F(fn, *a, **kw):
        return lambda: fn(*a, **kw)

    # ---- pre-stream ----
    kT_sb[0] = sb.tile([P, L], BF16, tag="kT", bufs=NPAIR, name="kT_sb")
    kT_sb[1] = sb.tile([P, L], BF16, tag="kT", bufs=NPAIR, name="kT_sb")
    qT[0] = sb.tile([P, L], BF16, tag="qT", bufs=NPAIR, name="qT")
    qT[1] = sb.tile([P, L], BF16, tag="qT", bufs=NPAIR, name="qT")
    new_pair(0)
    new_pair(1)
    qkv_chunk(kT_sb[0], 0, 0, 0)
    qkv_chunk(qT[0], 0, 1, 0)

    # ---- stream schedule: per position, dict beat -> [filler closures] ----
    sched = {}
    sched[(0, 0)] = {
        0: [F(qkv_chunk, kT_sb[0], 0, 0, 1)],
        2: [F(qkv_chunk, vT[0], 0, 2, 0)],
        4: [F(qkv_chunk, kT_sb[0], 0, 0, 2)],
        5: [F(emit_vtr, 0, [0, 1, 2, 3])],
        6: [F(qkv_chunk, vT[0], 0, 2, 1)],
        8: [F(qkv_chunk, kT_sb[0], 0, 0, 3)],
        9: [F(emit_vtr, 0, [4, 5, 6, 7])],
        10: [F(qkv_chunk, vT[0], 0, 2, 2)],
        11: [F(emit_vtr, 0, [8, 9, 10, 11])],
        12: [F(qkv_chunk, qT[0], 0, 1, 1)],
        13: [F(qkv_chunk, vT[0], 0, 2, 3)],
        14: [F(emit_vtr, 0, [12, 13, 14, 15])],
    }
    sched[(0, 1)] = {
        0: [F(att_open, 0, 0, 0), F(att_seg, 0, 0, 0, 4)],
        1: [F(att_seg, 0, 0, 0, 8)],
        2: [F(att_seg, 0, 0, 0, 12)],
        3: [F(att_close, 0, 0, 0)],
        4: [F(att_open, 0, 0, 1), F(att_seg, 0, 0, 1, 4)],
        5: [F(att_seg, 0, 0, 1, 8)],
        6: [F(att_seg, 0, 0, 1, 12)],
        7: [F(att_close, 0, 0, 1)],
        8: [F(att_open, 0, 1, 0), F(att_seg, 0, 1, 0, 6)],
        9: [F(att_seg, 0, 1, 0, 7)],
        10: [F(att_open, 0, 1, 1), F(att_seg, 0, 1, 1, 6)],
        11: [F(att_seg, 0, 1, 1, 7)],
        12: [F(att_seg, 0, 1, 0, 10), F(att_seg, 0, 1, 1, 10)],
        13: [F(att_seg, 0, 1, 0, 11), F(att_seg, 0, 1, 1, 11)],
        14: [F(att_seg, 0, 1, 0, 12), F(att_seg, 0, 1, 1, 12),
             F(qkv_chunk, qT[0], 0, 1, 2)],
        15: [F(att_seg, 0, 1, 0, 13), F(att_seg, 0, 1, 1, 13)],
    }

    def trail_sched(p, s, extra):
        """Self-trailing attend from beat 2 (chains of sub (p,s)), closing the
        previous sub's chains at beat 0, plus extra fillers."""
        d = {
            0: [F(att_close, p, s - 1, 0), F(att_close, p, s - 1, 1)],
            2: [F(att_open, p, s, 0), F(att_open, p, s, 1)],
        }
        for b in range(2, 16):
            d.setdefault(b, []).extend(
                [F(att_seg, p, s, 0, b - 1), F(att_seg, p, s, 1, b - 1)])
        for b, fs in extra.items():
            d.setdefault(b, []).extend(fs)
        return d

    sched[(0, 2)] = trail_sched(0, 2, {
        4: [F(qkv_chunk, kT_sb[1], 1, 0, 0)],
        8: [F(qkv_chunk, kT_sb[1], 1, 0, 1)],
        12: [F(qkv_chunk, qT[0], 0, 1, 3)],
    })
    sched[(0, 3)] = trail_sched(0, 3, {
        4: [F(qkv_chunk, kT_sb[1], 1, 0, 2)],
        8: [F(qkv_chunk, kT_sb[1], 1, 0, 3)],
        12: [F(qkv_chunk, qT[1], 1, 1, 0)],
        14: [F(qkv_chunk, vT[1], 1, 2, 0)],
    })
    sched[(1, 0)] = {
        0: [F(att_close, 0, 3, 0), F(att_close, 0, 3, 1)],
        1: [F(emit_vtr, 1, [0, 1, 2, 3])],
        2: [F(qkv_chunk, vT[1], 1, 2, 1)],
        4: [F(qkv_chunk, vT[1], 1, 2, 2)],
        6: [F(emit_vtr, 1, [4, 5, 6, 7])],
        7: [F(qkv_chunk, vT[1], 1, 2, 3)],
        9: [F(emit_vtr, 1, [8, 9, 10, 11])],
        10: [F(qkv_chunk, qT[1], 1, 1, 1)],
        12: [F(emit_vtr, 1, [12, 13, 14, 15])],
    }
    sched[(1, 1)] = {
        0: [F(att_open, 1, 0, 0), F(att_seg, 1, 0, 0, 4)],
        1: [F(att_seg, 1, 0, 0, 8)],
        2: [F(att_seg, 1, 0, 0, 12)],
        3: [F(att_close, 1, 0, 0)],
        4: [F(att_open, 1, 0, 1), F(att_seg, 1, 0, 1, 4)],
        5: [F(att_seg, 1, 0, 1, 8)],
        6: [F(att_seg, 1, 0, 1, 12)],
        7: [F(att_close, 1, 0, 1)],
        8: [F(att_open, 1, 1, 0), F(att_seg, 1, 1, 0, 6)],
        9: [F(att_seg, 1, 1, 0, 7)],
        10: [F(att_open, 1, 1, 1), F(att_seg, 1, 1, 1, 6)],
        11: [F(att_seg, 1, 1, 1, 7)],
        12: [F(att_seg, 1, 1, 0, 10), F(att_seg, 1, 1, 1, 10)],
        13: [F(att_seg, 1, 1, 0, 11), F(att_seg, 1, 1, 1, 11)],
        14: [F(att_seg, 1, 1, 0, 12), F(att_seg, 1, 1, 1, 12),
             F(qkv_chunk, qT[1], 1, 1, 2)],
        15: [F(att_seg, 1, 1, 0, 13), F(att_seg, 1, 1, 1, 13)],
    }
    sched[(1, 2)] = trail_sched(1, 2, {
        4: [F(emit_outproj_lt, 0)],
        6: [F(emit_outproj_lt, 1)],
        8: [F(emit_outproj_lt, 2), F(emit_ydma, 0, "gpsimd")],
        10: [F(emit_outproj_lt, 3)],
        12: [F(qkv_chunk, qT[1], 1, 1, 3), F(emit_ydma, 1, "sync")],
    })
    sched[(1, 3)] = trail_sched(1, 3, {
        4: [F(emit_outproj_lt, 4)],
        6: [F(emit_outproj_lt, 5)],
        8: [F(emit_outproj_lt, 6), F(emit_ydma, 2, "gpsimd")],
        10: [F(emit_outproj_lt, 7)],
        11: [F(emit_outproj_lt, 8), F(emit_ydma, 3, "sync")],
        12: [F(emit_outproj_lt, 9)],
        13: [F(emit_outproj_lt, 10), F(emit_ydma, 4, "gpsimd")],
        14: [F(emit_outproj_lt, 11)],
        15: [F(emit_ydma, 5, "sync")],
    })

    STREAM = [(0, 0), (0, 1), (0, 2), (0, 3), (1, 0), (1, 1), (1, 2), (1, 3)]
    for pos, (p, s) in enumerate(STREAM):
        dve_beats = DVE_BEATS.get((p, s), ())
        fillers = sched[(p, s)]
        for b in range(NLT):
            sp = emit_scores(p, s, b)
            emit_exp(p, s, b, sp, dve=(b in dve_beats))
            for f in fillers.get(b, ()):
                f()

    # ---- tail ----
    att_close(1, 3, 0)
    att_close(1, 3, 1)
    emit_outproj_lt(12, drain="scalar")
    emit_outproj_lt(13, drain="vector")
    emit_ydma(6, "sync")
    emit_outproj_lt(14, drain="scalar")
    emit_outproj_lt(15, drain="vector")
    emit_ydma(7, "scalar")


_NC_CACHE = {}


def _get_nc():
    if "nc" not in _NC_CACHE:
        _NC_CACHE["nc"] = build_nc()
    return _NC_CACHE["nc"]


def shard_inputs(x, Wq, bq, Wk, bk, Wv, bv, Wd, bd):
    """Build the 8 per-core input maps (host picks the on-device layout)."""
    in_maps = []
    x = np.asarray(x, np.float32)
    for c in range(NCORES):
        b = c // (NCORES // B)
        h0 = (c % (NCORES // B)) * H_PER_CORE
        hs = slice(h0, h0 + H_PER_CORE)
        # x packed [k, lc, kt, l'] bf16: contiguous 8KB DMA lines per chunk
        xT = (x[b].T.reshape(KT, P, NSUB, LCH).transpose(1, 2, 0, 3)
              .reshape(P, -1).astype(BF16_NP))
        xT = np.ascontiguousarray(xT)
        # wqkv bf16 [128, pp * KQV * kt * 128]: [k, pp, i, kt, n]
        ws = []
        for W in (Wk, Wq, Wv):
            w = np.asarray(W[:, hs, :], np.float32).reshape(DMODEL, 2 * P)
            ws.append(w.reshape(KT, P, NPAIR, P).transpose(1, 2, 0, 3))
        wqkv = np.stack(ws, axis=2).reshape(P, -1).astype(BF16_NP)
        # wd2 bf16 [128, pp * dmodel]: [k, pp, m]
        wd2 = (np.asarray(Wd[hs], np.float32).reshape(NPAIR, P, DMODEL)
               .transpose(1, 0, 2).reshape(P, -1).astype(BF16_NP))
        # biasp f32 [128, KQV * pp]: [k, i, pp]
        bs = [np.asarray(v[hs], np.float32).reshape(NPAIR, P).T
              for v in (bk, bq, bv)]
        biasp = np.ascontiguousarray(
            np.stack(bs, axis=1).reshape(P, -1))
        in_maps.append({
            "x": xT,
            "wqkv": np.ascontiguousarray(wqkv),
            "wd2": np.ascontiguousarray(wd2),
            "biasp": biasp,
        })
    return in_maps


def gather_outputs(results, bd):
    """Unshuffle the partition-major y, sum partials per batch, add bd."""
    out = np.zeros((B, L, DMODEL), np.float32)
    per_b = NCORES // B
    for c, res in enumerate(results):
        y2 = np.asarray(res["y"], np.float32)              # [128, NLT*DMODEL]
        y = y2.reshape(P, NLT, DMODEL).transpose(1, 0, 2).reshape(L, DMODEL)
        out[c // per_b] += y
    out += np.asarray(bd, np.float32)[None, None, :]
    return out


def kernel(x, Wq, bq, Wk, bk, Wv, bv, Wd, bd, _trace=False):
    nc = _get_nc()
    in_maps = shard_inputs(x, Wq, bq, Wk, bk, Wv, bv, Wd, bd)
    res = run_bass_kernel_spmd(nc, in_maps, list(range(NCORES)), trace=_trace)
    out = gather_outputs(res.results, bd)
    if _trace:
        kernel.last_results = res
    return out


# revision 9
# speedup vs baseline: 1.0850x; 1.0850x over previous
"""Trainium2 Bass kernel: multi-head attention (dense transformer block).

Computation (per batch b):
    Q = x @ Wq + bq ; K = x @ Wk + bk ; V = x @ Wv + bv        (per head)
    P = exp((Q @ K^T) / sqrt(Dh))                   (no max-subtraction needed:
                                                     scores are O(1) by construction)
    out = sum_h (P @ V / rowsum(P)) @ Wd[h] + bd

Sharding (data + tensor parallel): 8 cores; core c handles batch b = c // 4
and the 4 heads starting at 4*(c % 4). Each core computes a partial output
(bf16, in a [128, lt, m] partition-major DRAM layout for wide DMA lines);
the host unshuffles, sums the 4 partials per batch and adds bd.

Per-core dataflow: heads in 2 pairs stacked on partitions. Stream of 8
"positions" = (pair, sub) x 16 beats; each beat computes one l'-tile of
scores for both heads (PSUM, f32), then exp -> bf16 P-tile (ScalarE
mostly; a few beats per position go to DVE via a Schraudolph int16-bitcast
exp to keep DVE/ACT balanced where PE is light). Attend chains trail their
own sub's beats by ~2 (segments interleaved between beats); the softmax
denominator comes from ones-columns in the V tiles. Normalisation reads
the attend PSUM directly (reciprocal + multiply on DVE). Out-projection is
emitted per l-tile as soon as both pairs' norms for that sub are done, and
the output streams to DRAM in 2-l-tile batches round-robined over the
sync/scalar/gpsimd DMA queues so almost nothing is exposed at the tail.
"""

import os
import sys
from contextlib import ExitStack

import ml_dtypes
import numpy as np

for _p in ("/opt/trn_rl_repo", "/root/.axon_site/_ro/trn_rl_repo"):
    if os.path.isdir(_p) and _p not in sys.path:
        sys.path.append(_p)

import concourse.bass as bass
import concourse.tile as tile
from concourse import bacc, mybir
from concourse.bass import ds
from concourse.bass_utils import run_bass_kernel_spmd
from concourse.masks import make_identity

F32 = mybir.dt.float32
BF16 = mybir.dt.bfloat16
I16 = mybir.dt.int16
BF16_NP = ml_dtypes.bfloat16

# Problem sizes (hardcoded per contract).
DMODEL, HEADS, DHEAD = 1024, 16, 64
B, L = 2, 2048
NCORES = 8
H_PER_CORE = B * HEADS // NCORES          # 4 heads per core
NPAIR = H_PER_CORE // 2                   # head pairs per core
P = 128                                   # partitions
KT = DMODEL // P                          # 8 k-tiles over dmodel
NLT = L // P                              # 16 l-tiles
LCH = 512                                 # sub-chunk width (one psum bank pair)
NSUB = L // LCH                           # 4 sub-chunks
MCH = 512                                 # m-chunk for out-proj
NMC = DMODEL // MCH
PT_BUFS = 40                              # P^T tiles in flight
WCH = 3 * KT * P                          # per-pair packed qkv weight columns
SCALE = 1.0 / np.sqrt(DHEAD)

# Schraudolph exp via int16 bitcast to bf16: i16 = round(x*SCH_A + SCH_B),
# bits(i16) ~ bf16(exp(x*SCALE)).  C=5.6 centers the relative error (~±3%).
SCH_A = (2.0 ** 7 / np.log(2.0)) * SCALE
SCH_B = 127.0 * 2 ** 7 - 5.6

# Beats whose exp runs on DVE instead of ScalarE, per stream position.
DVE_BEATS = {
    (1, 0): (6, 10, 14),
}

EXPF = mybir.ActivationFunctionType.Exp
QUAD_SCORES = False


def build_nc():
    """Build the SPMD Bass program for one core."""
    nc = bacc.Bacc("TRN2", target_bir_lowering=False, debug=False,
                   num_devices=NCORES)

    # Pre-packed per-core inputs (see shard_inputs):
    #   x:    x^T bf16 [dmodel, L] packed [k, sub, kt, l']
    #   wqkv: bf16 [128, pp(2) * i(3: K,Q,V) * kt(8) * 128]
    #   wd2:  bf16 [128, pp(2) * dmodel]
    #   biasp: f32 [128, i(3: K,Q,V) * pp(2)]
    x_d = nc.dram_tensor("x", [P, NSUB * KT * LCH], BF16, kind="ExternalInput").ap()
    wqkv_d = nc.dram_tensor("wqkv", [P, NPAIR * WCH], BF16, kind="ExternalInput").ap()
    wd2_d = nc.dram_tensor("wd2", [P, NPAIR * DMODEL], BF16, kind="ExternalInput").ap()
    biasp_d = nc.dram_tensor("biasp", [P, 3 * NPAIR], F32, kind="ExternalInput").ap()
    # y partition-major: y2[p, lt*DMODEL + m] = y[lt*128 + p, m]
    y_d = nc.dram_tensor("y", [P, NLT * DMODEL], BF16, kind="ExternalOutput").ap()

    with ExitStack() as ctx:
        tc = ctx.enter_context(tile.TileContext(nc))
        _body(nc, tc, ctx, x_d, wqkv_d, wd2_d, biasp_d, y_d)
    nc.compile()
    return nc


def _body(nc, tc, ctx, x_d, wqkv_d, wd2_d, biasp_d, y_d):
    const = ctx.enter_context(tc.tile_pool(name="const", bufs=1))
    sb = ctx.enter_context(tc.tile_pool(name="sb", bufs=1))
    psum = ctx.enter_context(tc.tile_pool(name="psum", bufs=1, space="PSUM"))

    # ACT exp-table warmup off the critical path
    warm = const.tile([P, 1], BF16)
    zz = const.tile([P, 1], F32)
    nc.vector.memset(zz, 0.0)
    nc.scalar.activation(warm, zz, func=EXPF)

    # PE identity (for V transposes) + HAM warm-up matmuls on junk data.
    ident = const.tile([P, P], BF16)
    make_identity(nc, ident)
    for _ in range(6):
        jp = psum.tile([P, 2, P], F32, tag="qkvp", bufs=2, name="jp")
        nc.tensor.matmul(jp[:, 0], lhsT=ident, rhs=ident, start=True, stop=True)
        nc.tensor.matmul(jp[:, 1], lhsT=ident, rhs=ident, start=True, stop=True)

    # ---- input DMAs: weights on the scalar HWDGE queue, x on sync ----
    w_sb = const.tile([P, NPAIR, 3, KT, P], BF16)    # [k, pp, KQV, kt, n]
    wd_sb = const.tile([P, NPAIR, DMODEL], BF16)
    wf = w_sb.rearrange("k pp i kt n -> k (pp i kt n)")
    xt = sb.tile([P, NSUB, KT, LCH], BF16)
    xtf = xt.rearrange("k c kt l -> k (c kt l)")
    CB = KT * LCH
    nc.scalar.dma_start(wf[:, 0:2 * KT * P], wqkv_d[:, 0:2 * KT * P])   # K0+Q0
    nc.sync.dma_start(xtf[:, ds(0, CB // 2)], x_d[:, ds(0, CB // 2)])
    nc.sync.dma_start(xtf[:, ds(CB // 2, CB // 2)], x_d[:, ds(CB // 2, CB // 2)])
    nc.scalar.dma_start(wf[:, ds(2 * KT * P, KT * P)],
                        wqkv_d[:, ds(2 * KT * P, KT * P)])              # V0
    nc.sync.dma_start(xtf[:, ds(CB, CB)], x_d[:, ds(CB, CB)])
    nc.scalar.dma_start(wf[:, ds(WCH, WCH)], wqkv_d[:, ds(WCH, WCH)])   # pair 1
    nc.sync.dma_start(xtf[:, ds(2 * CB, CB)], x_d[:, ds(2 * CB, CB)])
    nc.scalar.dma_start(wd_sb.rearrange("k pp m -> k (pp m)"), wd2_d)
    nc.sync.dma_start(xtf[:, ds(3 * CB, CB)], x_d[:, ds(3 * CB, CB)])

    bias_sb = const.tile([P, 3, NPAIR], F32)
    nc.gpsimd.dma_start(bias_sb.rearrange("k i p -> k (i p)"), biasp_d)

    o_norm = sb.tile([P, NPAIR, L], BF16)

    # per-pair state
    kT_sb = [None] * NPAIR
    qT = [None] * NPAIR
    vT = [None] * NPAIR
    vt = [None] * NPAIR
    pt_tiles = [[[None] * NLT for _ in range(NSUB)] for _ in range(NPAIR)]

    qkv_ps = {}

    def qkv_half(dst, p, i, lc, half):
        """Half of a qkv chunk (kt 0-3 or 4-7); drain after the second half."""
        if half == 0:
            qkv_ps[(p, i, lc)] = psum.tile([P, LCH], F32, tag="qkvp", bufs=2,
                                           name="qkvps")
        ps = qkv_ps[(p, i, lc)]
        for kt in range(4 * half, 4 * half + 4):
            nc.tensor.matmul(
                ps, lhsT=w_sb[:, p, i, kt],
                rhs=xt[:, lc, kt, :],
                start=(kt == 0), stop=(kt == KT - 1),
                skip_group_check=True)
        if half == 1:
            nc.vector.tensor_scalar_add(
                dst[:, ds(lc * LCH, LCH)], qkv_ps.pop((p, i, lc)),
                bias_sb[:, i, p:p + 1])

    def qkv_chunk(dst, p, i, lc):
        qkv_half(dst, p, i, lc, 0)
        qkv_half(dst, p, i, lc, 1)

    def emit_vtr(p, lts):
        """PE-transpose V^T l-tiles into vt[:, lt] = [1*64|V_h0|1*64|V_h1]."""
        for lt in lts:
            tp = psum.tile([P, P], BF16, tag="qkvp", bufs=2, name="tp")
            nc.tensor.transpose(tp, vT[p][:, ds(lt * P, P)], ident)
            out3 = vt[p][:, lt].rearrange("p (i n) -> p i n", n=P)[:, :, DHEAD:P]
            nc.vector.tensor_copy(
                out3, tp.rearrange("p (i n) -> p i n", n=DHEAD))

    def emit_scores(p, s, lt):
        sp = psum.tile([P, 2, LCH], F32, tag="sctr", bufs=2, name="sp")
        if QUAD_SCORES:
            for h in range(2):
                for cq in range(2):
                    nc.tensor.matmul(
                        sp[ds(64 * cq, 64), h, :],
                        lhsT=kT_sb[p][ds(64 * h, 64), ds(lt * P + 64 * cq, 64)],
                        rhs=qT[p][ds(64 * h, 64), ds(s * LCH, LCH)],
                        start=True, stop=True,
                        tile_position=(64 * h, 64 * cq))
        else:
            for h in range(2):
                nc.tensor.matmul(
                    sp[:, h],
                    lhsT=kT_sb[p][ds(64 * h, 64), ds(lt * P, P)],
                    rhs=qT[p][ds(64 * h, 64), ds(s * LCH, LCH)],
                    start=True, stop=True)
        return sp

    def emit_exp(p, s, lt, sp, dve):
        pt = sb.tile([P, 2, LCH], BF16, tag="pt", bufs=PT_BUFS, name="pt")
        src = sp.rearrange("p a b -> p (a b)")
        dst = pt.rearrange("p a b -> p (a b)")
        if dve:
            nc.vector.tensor_scalar(
                dst.bitcast(I16), src, float(SCH_A), float(SCH_B),
                op0=mybir.AluOpType.mult, op1=mybir.AluOpType.add)
        else:
            nc.scalar.activation(dst, src, func=EXPF, scale=float(SCALE))
        pt_tiles[p][s][lt] = pt

    # ---- attend chain bookkeeping ----
    chains = {}

    def att_open(p, s, h):
        chains[(p, s, h)] = {
            "op": psum.tile([P, LCH], F32, tag="op", bufs=2, name="op"),
            "lt": 0}

    def att_seg(p, s, h, upto):
        c = chains[(p, s, h)]
        for lt in range(c["lt"], upto):
            nc.tensor.matmul(
                c["op"], lhsT=vt[p][:, lt, ds(P * h, P)],
                rhs=pt_tiles[p][s][lt][:, h, :],
                start=(lt == 0), stop=(lt == NLT - 1),
                skip_group_check=True)
        c["lt"] = upto

    def att_close(p, s, h):
        att_seg(p, s, h, NLT)
        op = chains.pop((p, s, h))["op"]
        rs = sb.tile([DHEAD, LCH], F32, tag="rs", bufs=2)
        nc.vector.reciprocal_approx_fast(rs, op[0:DHEAD, :])
        nc.vector.tensor_mul(
            o_norm[ds(64 * h, 64), p, ds(s * LCH, LCH)], op[DHEAD:P, :], rs)

    # ---- out-projection + output DMA ----
    ysb_tiles = {}

    def emit_outproj_lt(lt, drain="vector"):
        pr = lt // 2
        if pr not in ysb_tiles:
            ysb_tiles[pr] = sb.tile([P, 2, DMODEL], BF16, tag="ysb", bufs=3,
                                    name="ysb")
        yb = ysb_tiles[pr]
        yps = [psum.tile([P, MCH], F32, tag="qkvp", bufs=2, name="yp")
               for _ in range(NMC)]
        for p in range(NPAIR):
            for mc in range(NMC):
                nc.tensor.matmul(
                    yps[mc], lhsT=o_norm[:, p, ds(lt * P, P)],
                    rhs=wd_sb[:, p, ds(mc * MCH, MCH)],
                    start=(p == 0), stop=(p == NPAIR - 1),
                    skip_group_check=True)
        for mc in range(NMC):
            dstc = yb[:, lt % 2, ds(mc * MCH, MCH)]
            if drain == "scalar":
                nc.scalar.copy(dstc, yps[mc])
            else:
                nc.vector.tensor_copy(dstc, yps[mc])

    def emit_ydma(pr, queue):
        yb = ysb_tiles.pop(pr)
        dst = y_d[:, ds(pr * 2 * DMODEL, 2 * DMODEL)]
        eng = {"sync": nc.sync, "scalar": nc.scalar, "gpsimd": nc.gpsimd}[queue]
        eng.dma_start(dst, yb.rearrange("p a b -> p (a b)"))

    def new_pair(p):
        vT[p] = sb.tile([P, L], BF16, tag="vT", bufs=1, name="vT")
        vt[p] = sb.tile([P, NLT, 2 * P], BF16, tag="vt", bufs=NPAIR, name="vt")
        nc.vector.memset(vt[p][:, :, 0:DHEAD], 1.0)
        nc.vector.memset(vt[p][:, :, P:P + DHEAD], 1.0)

    def F(fn, *a, **kw):
        return lambda: fn(*a, **kw)

    # ---- pre-stream ----
    kT_sb[0] = sb.tile([P, L], BF16, tag="kT", bufs=NPAIR, name="kT_sb")
    kT_sb[1] = sb.tile([P, L], BF16, tag="kT", bufs=NPAIR, name="kT_sb")
    qT[0] = sb.tile([P, L], BF16, tag="qT", bufs=NPAIR, name="qT")
    qT[1] = sb.tile([P, L], BF16, tag="qT", bufs=NPAIR, name="qT")
    new_pair(0)
    new_pair(1)
    qkv_chunk(kT_sb[0], 0, 0, 0)
    qkv_chunk(qT[0], 0, 1, 0)

    # ---- stream schedule: per position, dict beat -> [filler closures] ----
    sched = {}
    sched[(0, 0)] = {
        0: [F(qkv_half, kT_sb[0], 0, 0, 1, 0)],
        1: [F(qkv_half, kT_sb[0], 0, 0, 1, 1)],
        2: [F(qkv_half, vT[0], 0, 2, 0, 0)],
        3: [F(qkv_half, vT[0], 0, 2, 0, 1)],
        4: [F(qkv_half, kT_sb[0], 0, 0, 2, 0)],
        5: [F(qkv_half, kT_sb[0], 0, 0, 2, 1), F(emit_vtr, 0, [0, 1, 2, 3])],
        6: [F(qkv_half, vT[0], 0, 2, 1, 0)],
        7: [F(qkv_half, vT[0], 0, 2, 1, 1)],
        8: [F(qkv_half, kT_sb[0], 0, 0, 3, 0)],
        9: [F(qkv_half, kT_sb[0], 0, 0, 3, 1), F(emit_vtr, 0, [4, 5, 6, 7])],
        10: [F(qkv_half, qT[0], 0, 1, 1, 0)],
        11: [F(qkv_half, qT[0], 0, 1, 1, 1)],
        12: [F(qkv_half, vT[0], 0, 2, 2, 0)],
        13: [F(qkv_half, vT[0], 0, 2, 2, 1), F(emit_vtr, 0, [8, 9, 10, 11])],
        14: [F(qkv_half, vT[0], 0, 2, 3, 0)],
        15: [F(qkv_half, vT[0], 0, 2, 3, 1), F(emit_vtr, 0, [12, 13, 14, 15])],
    }
    sched[(0, 1)] = {
        0: [F(att_open, 0, 0, 0), F(att_seg, 0, 0, 0, 4)],
        1: [F(att_seg, 0, 0, 0, 8)],
        2: [F(att_seg, 0, 0, 0, 12)],
        3: [F(att_close, 0, 0, 0)],
        4: [F(att_open, 0, 0, 1), F(att_seg, 0, 0, 1, 4)],
        5: [F(att_seg, 0, 0, 1, 8)],
        6: [F(att_seg, 0, 0, 1, 12)],
        7: [F(att_close, 0, 0, 1)],
        8: [F(att_open, 0, 1, 0), F(att_seg, 0, 1, 0, 6)],
        9: [F(att_seg, 0, 1, 0, 7)],
        10: [F(att_open, 0, 1, 1), F(att_seg, 0, 1, 1, 6)],
        11: [F(att_seg, 0, 1, 1, 7)],
        12: [F(att_seg, 0, 1, 0, 10), F(att_seg, 0, 1, 1, 10)],
        13: [F(att_seg, 0, 1, 0, 11), F(att_seg, 0, 1, 1, 11)],
        11: [F(qkv_half, qT[0], 0, 1, 2, 0)],
        12: [F(qkv_half, qT[0], 0, 1, 2, 1)],
        14: [F(att_seg, 0, 1, 0, 12), F(att_seg, 0, 1, 1, 12)],
        15: [F(att_seg, 0, 1, 0, 13), F(att_seg, 0, 1, 1, 13)],
    }

    def trail_sched(p, s, extra):
        """Self-trailing attend from beat 2 (chains of sub (p,s)), closing the
        previous sub's chains at beat 0, plus extra fillers."""
        d = {
            0: [F(att_close, p, s - 1, 0), F(att_close, p, s - 1, 1)],
            2: [F(att_open, p, s, 0), F(att_open, p, s, 1)],
        }
        for b in range(2, 16):
            d.setdefault(b, []).extend(
                [F(att_seg, p, s, 0, b - 1), F(att_seg, p, s, 1, b - 1)])
        for b, fs in extra.items():
            d.setdefault(b, []).extend(fs)
        return d

    sched[(0, 2)] = trail_sched(0, 2, {
        3: [F(qkv_half, kT_sb[1], 1, 0, 0, 0)],
        4: [F(qkv_half, kT_sb[1], 1, 0, 0, 1)],
        7: [F(qkv_half, kT_sb[1], 1, 0, 1, 0)],
        8: [F(qkv_half, kT_sb[1], 1, 0, 1, 1)],
        10: [F(qkv_half, qT[0], 0, 1, 3, 0)],
        11: [F(qkv_half, qT[0], 0, 1, 3, 1)],
    })
    sched[(0, 3)] = trail_sched(0, 3, {
        2: [F(qkv_half, kT_sb[1], 1, 0, 2, 0)],
        3: [F(qkv_half, kT_sb[1], 1, 0, 2, 1)],
        5: [F(qkv_half, kT_sb[1], 1, 0, 3, 0)],
        6: [F(qkv_half, kT_sb[1], 1, 0, 3, 1)],
        8: [F(qkv_half, qT[1], 1, 1, 0, 0)],
        9: [F(qkv_half, qT[1], 1, 1, 0, 1)],
        11: [F(qkv_half, vT[1], 1, 2, 0, 0)],
        12: [F(qkv_half, vT[1], 1, 2, 0, 1)],
    })
    sched[(1, 0)] = {
        0: [F(att_close, 0, 3, 0), F(att_close, 0, 3, 1)],
        1: [F(emit_vtr, 1, [0, 1, 2, 3])],
        2: [F(qkv_half, vT[1], 1, 2, 1, 0)],
        3: [F(qkv_half, vT[1], 1, 2, 1, 1)],
        4: [F(qkv_half, vT[1], 1, 2, 2, 0)],
        5: [F(qkv_half, vT[1], 1, 2, 2, 1)],
        6: [F(emit_vtr, 1, [4, 5, 6, 7])],
        7: [F(qkv_half, vT[1], 1, 2, 3, 0)],
        8: [F(qkv_half, vT[1], 1, 2, 3, 1)],
        9: [F(qkv_half, qT[1], 1, 1, 1, 0)],
        10: [F(qkv_half, qT[1], 1, 1, 1, 1), F(emit_vtr, 1, [8, 9, 10, 11])],
        12: [F(emit_vtr, 1, [12, 13, 14, 15])],
    }
    sched[(1, 1)] = {
        0: [F(att_open, 1, 0, 0), F(att_seg, 1, 0, 0, 4)],
        1: [F(att_seg, 1, 0, 0, 8)],
        2: [F(att_seg, 1, 0, 0, 12)],
        3: [F(att_close, 1, 0, 0)],
        4: [F(att_open, 1, 0, 1), F(att_seg, 1, 0, 1, 4)],
        5: [F(att_seg, 1, 0, 1, 8)],
        6: [F(att_seg, 1, 0, 1, 12)],
        7: [F(att_close, 1, 0, 1)],
        8: [F(att_open, 1, 1, 0), F(att_seg, 1, 1, 0, 6)],
        9: [F(att_seg, 1, 1, 0, 7)],
        10: [F(att_open, 1, 1, 1), F(att_seg, 1, 1, 1, 6)],
        11: [F(att_seg, 1, 1, 1, 7)],
        12: [F(att_seg, 1, 1, 0, 10), F(att_seg, 1, 1, 1, 10)],
        13: [F(att_seg, 1, 1, 0, 11), F(att_seg, 1, 1, 1, 11)],
        11: [F(qkv_half, qT[1], 1, 1, 2, 0)],
        12: [F(qkv_half, qT[1], 1, 1, 2, 1)],
        14: [F(att_seg, 1, 1, 0, 12), F(att_seg, 1, 1, 1, 12)],
        15: [F(att_seg, 1, 1, 0, 13), F(att_seg, 1, 1, 1, 13)],
    }
    sched[(1, 2)] = trail_sched(1, 2, {
        3: [F(emit_outproj_lt, 0)],
        5: [F(emit_outproj_lt, 1)],
        7: [F(emit_outproj_lt, 2), F(emit_ydma, 0, "gpsimd")],
        9: [F(emit_outproj_lt, 3)],
        11: [F(qkv_half, qT[1], 1, 1, 3, 0)],
        12: [F(qkv_half, qT[1], 1, 1, 3, 1), F(emit_ydma, 1, "sync")],
    })
    sched[(1, 3)] = trail_sched(1, 3, {
        3: [F(emit_outproj_lt, 4)],
        5: [F(emit_outproj_lt, 5)],
        7: [F(emit_outproj_lt, 6), F(emit_ydma, 2, "gpsimd")],
        9: [F(emit_outproj_lt, 7)],
        10: [F(emit_outproj_lt, 8), F(emit_ydma, 3, "sync")],
        12: [F(emit_outproj_lt, 9), F(emit_ydma, 4, "gpsimd")],
    })

    STREAM = [(0, 0), (0, 1), (0, 2), (0, 3), (1, 0), (1, 1), (1, 2), (1, 3)]
    for pos, (p, s) in enumerate(STREAM):
        dve_beats = DVE_BEATS.get((p, s), ())
        fillers = sched[(p, s)]
        for b in range(NLT):
            sp = emit_scores(p, s, b)
            emit_exp(p, s, b, sp, dve=(b in dve_beats))
            for f in fillers.get(b, ()):
                f()

    # ---- tail ----
    att_close(1, 3, 0)
    att_close(1, 3, 1)
    emit_outproj_lt(10, drain="scalar")
    emit_outproj_lt(11, drain="scalar")
    emit_ydma(5, "sync")
    emit_outproj_lt(12, drain="scalar")
    emit_outproj_lt(13, drain="vector")
    emit_ydma(6, "gpsimd")
    emit_outproj_lt(14, drain="scalar")
    emit_outproj_lt(15, drain="vector")
    emit_ydma(7, "scalar")


_NC_CACHE = {}


def _get_nc():
    if "nc" not in _NC_CACHE:
        _NC_CACHE["nc"] = build_nc()
    return _NC_CACHE["nc"]


def shard_inputs(x, Wq, bq, Wk, bk, Wv, bv, Wd, bd):
    """Build the 8 per-core input maps (host picks the on-device layout)."""
    in_maps = []
    x = np.asarray(x, np.float32)
    for c in range(NCORES):
        b = c // (NCORES // B)
        h0 = (c % (NCORES // B)) * H_PER_CORE
        hs = slice(h0, h0 + H_PER_CORE)
        # x packed [k, lc, kt, l'] bf16: contiguous 8KB DMA lines per chunk
        xT = (x[b].T.reshape(KT, P, NSUB, LCH).transpose(1, 2, 0, 3)
              .reshape(P, -1).astype(BF16_NP))
        xT = np.ascontiguousarray(xT)
        # wqkv bf16 [128, pp * KQV * kt * 128]: [k, pp, i, kt, n]
        ws = []
        for W in (Wk, Wq, Wv):
            w = np.asarray(W[:, hs, :], np.float32).reshape(DMODEL, 2 * P)
            ws.append(w.reshape(KT, P, NPAIR, P).transpose(1, 2, 0, 3))
        wqkv = np.stack(ws, axis=2).reshape(P, -1).astype(BF16_NP)
        # wd2 bf16 [128, pp * dmodel]: [k, pp, m]
        wd2 = (np.asarray(Wd[hs], np.float32).reshape(NPAIR, P, DMODEL)
               .transpose(1, 0, 2).reshape(P, -1).astype(BF16_NP))
        # biasp f32 [128, KQV * pp]: [k, i, pp]
        bs = [np.asarray(v[hs], np.float32).reshape(NPAIR, P).T
              for v in (bk, bq, bv)]
        biasp = np.ascontiguousarray(
            np.stack(bs, axis=1).reshape(P, -1))
        in_maps.append({
            "x": xT,
            "wqkv": np.ascontiguousarray(wqkv),
            "wd2": np.ascontiguousarray(wd2),
            "biasp": biasp,
        })
    return in_maps


def gather_outputs(results, bd):
    """Unshuffle the partition-major y, sum partials per batch, add bd."""
    out = np.zeros((B, L, DMODEL), np.float32)
    per_b = NCORES // B
    for c, res in enumerate(results):
        y2 = np.asarray(res["y"], np.float32)              # [128, NLT*DMODEL]
        y = y2.reshape(P, NLT, DMODEL).transpose(1, 0, 2).reshape(L, DMODEL)
        out[c // per_b] += y
    out += np.asarray(bd, np.float32)[None, None, :]
    return out


def kernel(x, Wq, bq, Wk, bk, Wv, bv, Wd, bd, _trace=False):
    nc = _get_nc()
    in_maps = shard_inputs(x, Wq, bq, Wk, bk, Wv, bv, Wd, bd)
    res = run_bass_kernel_spmd(nc, in_maps, list(range(NCORES)), trace=_trace)
    out = gather_outputs(res.results, bd)
    if _trace:
        kernel.last_results = res
    return out
